# revision 85
# baseline (speedup 1.0000x reference)
"""Causal linear attention (elu+1 feature map) for Trainium2, 8 NeuronCores.

Problem: B=2, S=2048, D=1024, H=16, HD=64.
  q/k/v projections [S,D]@[D,H*HD], phi = elu+1, causal linear attention
  out[t] = (sum_{i<=t} (phi_q[t].phi_k[i]) v[i]) / (phi_q[t].sum_{i<=t} phi_k[i] + eps)

Sharding: core c -> (batch b=c//4, heads h0=4*(c%4) .. h0+3). No cross-core comm.
Host feeds x^T [D,S] per core in fp16 so the contraction dim d sits on SBUF
partitions with no on-chip transposes; all matmul operands are fp16 (fp32 PSUM
accumulate) which runs the PE at 1 cycle/row and halves HBM traffic.

Device algorithm (per core, 4 heads):
  - proj q,k -> phi_qT/phi_kT [64,2048] per head (head-pairs packed on 128
    partitions); v projected seq-major with an appended ones column whose
    matmul image is the softmax-free normalizer qz
  - phi_k seq layout via 128x128 PE transposes (matmul with identity moving
    operand, so the PSUM result stays fp32)
  - chunked attention, L=128, with two HW constraints found empirically:
    gpsimd cannot touch PSUM, and matmuls whose operands sit in different PE
    quadrants (partitions 0:64 vs 64:128) must not share a PSUM bank. Each
    chunk therefore uses two parity PSUM tiles [128, A 2x128 | out 2x65 with
    the sinc region overlapping A]: heads 0,2 (quadrant 0) in one, heads 1,3
    (quadrant 1) in the other. Causal mask is a fused tensor_tensor per
    parity on DVE; normalize = strided reciprocal (DVE) + scaled copies
    split between Act (activation Copy with per-partition scale) and DVE.
  - emission is software-pipelined: attention stage units interleave into
    the projection stream one-per-slot a few blocks behind their phi/v
    producers; projection groups are uneven (4,4,4,3,1 chunks) so the final
    phi block is tiny and the post-projection tail is ~one chunk.
"""

import threading

import numpy as np

B, S, D, H, HD = 2, 2048, 1024, 16, 64
N_CORES = 8
HPC = 4            # heads per core
HDC = HPC * HD     # 256 projected cols per core
NCHUNK = S // 128  # 16

_lock = threading.Lock()
_cache = {}


def _build_nc():
    import concourse.bass as bass
    import concourse.tile as tile
    from concourse import bacc, mybir

    f32 = mybir.dt.float32
    f16 = mybir.dt.float16
    Alu = mybir.AluOpType
    Act = mybir.ActivationFunctionType

    nc = bacc.Bacc("TRN2", target_bir_lowering=False, debug=False)

    xqT = nc.dram_tensor("xqT", [D, S], f16, kind="ExternalInput").ap()
    xkT = nc.dram_tensor("xkT", [D, S], f16, kind="ExternalInput").ap()
    xvT = nc.dram_tensor("xvT", [D, S], f16, kind="ExternalInput").ap()
    wq = nc.dram_tensor("wq", [D, HDC], f16, kind="ExternalInput").ap()
    wk = nc.dram_tensor("wk", [D, HDC], f16, kind="ExternalInput").ap()
    wv = nc.dram_tensor("wv", [D, HDC], f16, kind="ExternalInput").ap()
    out = nc.dram_tensor("out", [S, HDC], f16, kind="ExternalOutput").ap()

    DC = D // 128  # 8 contraction chunks

    with tile.TileContext(nc) as tc:
        with (
            tc.tile_pool(name="consts", bufs=1) as consts,
            tc.tile_pool(name="weights", bufs=1) as wpool,
            tc.tile_pool(name="resident", bufs=1) as res,
            tc.tile_pool(name="xin", bufs=1) as xin,
            tc.tile_pool(name="work", bufs=5) as work,
            tc.tile_pool(name="attn", bufs=3) as attn,
            tc.tile_pool(name="psum", bufs=2, space="PSUM") as psum,
        ):
            # ---- constants ----
            ones = consts.tile([128, 128], f16)
            nc.vector.memset(ones[:], 1.0)
            ident = consts.tile([128, 128], f16)
            nc.gpsimd.affine_select(
                ident[:], ones[:], pattern=[[-1, 128]], base=0,
                channel_multiplier=1, compare_op=Alu.is_equal, fill=0.0,
            )
            # causal mask in [j (part), t (free)] layout, 4 copies side by side
            maskT4 = consts.tile([128, 4, 128], f16)
            for i in range(4):
                nc.gpsimd.affine_select(
                    maskT4[:, i, :], ones[:], pattern=[[1, 128]], base=0,
                    channel_multiplier=-1, compare_op=Alu.is_ge, fill=0.0,
                )

            # ---- weights: [D, HDC] -> [128, DC, HDC] (partition = d % 128) ----
            # (DMA issued below, interleaved with the quarter-0 x loads)
            w_sb = {}
            w_dram = {"q": wq, "k": wk, "v": wv}
            for name in ("q", "k", "v"):
                w_sb[name] = wpool.tile([128, DC, HDC], f16, name=f"w{name}_sb")

            # ---- resident activations ----
            # head pairs hp=0 (heads 0,1) / hp=1 (heads 2,3), head at partition 64*(h%2)
            phi_qT = [res.tile([128, S], f16, name=f"phi_qT{i}") for i in range(2)]
            phi_kT = [res.tile([128, S], f16, name=f"phi_kT{i}") for i in range(2)]
            # seq-major: [s-in-chunk, (chunk, head, :)]
            phi_ks = res.tile([128, NCHUNK, HDC], f16, name="phi_ks")
            v_aug = res.tile([128, NCHUNK, HPC, 65], f16, name="v_aug")
            nc.vector.memset(v_aug[:, :, :, 64:65], 1.0)

            # ---- all input DMAs issued up front (SP queue), fp16 ----
            # x^T tiles: [128, DC, 512] per quarter, split in two for earlier
            # start; quarter 0 interleaved with the weight loads above is
            # handled by emission order (wq/xq0 first).
            xt = {}
            for qt in range(4):
                for tname, xdram in (("q", xqT), ("k", xkT), ("v", xvT)):
                    t = xin.tile([128, DC, 512], f16, name=f"x_{tname}_{qt}")
                    xt[(tname, qt)] = t

            def load_x(tname, qt, nsplit):
                t = xt[(tname, qt)]
                xdram = {"q": xqT, "k": xkT, "v": xvT}[tname]
                src = xdram.rearrange("(dc p) s -> p dc s", p=128)
                step = DC // nsplit
                for i in range(nsplit):
                    nc.sync.dma_start(
                        t[:, i * step:(i + 1) * step, :],
                        src[:, i * step:(i + 1) * step,
                            qt * 512:(qt + 1) * 512])

            # weight loads interleaved with quarter-0 x loads, dc-chunk by
            # dc-chunk for tensor "q" so the first projection matmul can start
            # as early as possible
            wsrc = {t: w_dram[t].rearrange("(dc p) m -> p dc m", p=128)
                    for t in ("q", "k", "v")}
            xsrc = {t: d.rearrange("(dc p) s -> p dc s", p=128)
                    for t, d in (("q", xqT), ("k", xkT), ("v", xvT))}
            for i in range(4):
                sl = slice(2 * i, 2 * i + 2)
                nc.sync.dma_start(w_sb["q"][:, sl, :], wsrc["q"][:, sl, :])
                nc.sync.dma_start(xt[("q", 0)][:, sl, :],
                                  xsrc["q"][:, sl, 0:512])
            for tname in ("k", "v"):
                nc.sync.dma_start(w_sb[tname][:, 0:DC // 2, :],
                                  wsrc[tname][:, 0:DC // 2, :])
                nc.sync.dma_start(w_sb[tname][:, DC // 2:DC, :],
                                  wsrc[tname][:, DC // 2:DC, :])
                load_x(tname, 0, 4)
            for qt in range(1, 4):
                for tname in ("q", "k", "v"):
                    load_x(tname, qt, 2)

            def phi_from_psum(ps, dst, w):
                # phi(x) = exp(min(x,0)) + max(x,0); m' = relu(-x); e = exp(-m')
                t1 = work.tile([128, 512], f32, tag="phi1")
                t2 = work.tile([128, 512], f16, tag="phi2")
                nc.scalar.activation(t1[:, 0:w], ps[:, 0:w],
                                     Act.Relu, scale=-1.0)
                nc.scalar.activation(t2[:, 0:w], t1[:, 0:w],
                                     Act.Exp, scale=-1.0)
                nc.vector.scalar_tensor_tensor(
                    dst, ps[:, 0:w], 0.0, t2[:, 0:w],
                    op0=Alu.max, op1=Alu.add)

            def emit_proj(tname, c0, n):
                # project chunks [c0, c0+n) of tensor q/k (all within one
                # quarter's resident x tile)
                qt, cc0 = c0 // 4, (c0 % 4) * 128
                w = n * 128
                dst = {"q": phi_qT, "k": phi_kT}[tname]
                x = xt[(tname, qt)]
                for hp in range(2):
                    ps = psum.tile([128, 512], f32, tag="proj",
                                   name=f"ps_{tname}_{c0}_{hp}", bufs=3)
                    for dc in range(DC):
                        nc.tensor.matmul(
                            ps[:, 0:w],
                            w_sb[tname][:, dc, hp * 128:(hp + 1) * 128],
                            x[:, dc, cc0:cc0 + w],
                            start=(dc == 0), stop=(dc == DC - 1),
                        )
                    phi_from_psum(ps, dst[hp][:, c0 * 128:c0 * 128 + w], w)

            def emit_proj_v_chunk(c):
                qt, cc = c // 4, c % 4
                xv = xt[("v", qt)]
                ps = psum.tile([128, 512], f32, tag="proj", name=f"ps_v_{c}",
                               bufs=3)
                for dc in range(DC):
                    nc.tensor.matmul(
                        ps[:, 0:HDC], xv[:, dc, cc * 128:(cc + 1) * 128],
                        w_sb["v"][:, dc, :],
                        start=(dc == 0), stop=(dc == DC - 1),
                    )
                nc.scalar.activation(
                    v_aug[:, c, :, 0:64],
                    ps[:, 0:HDC].rearrange("p (h e) -> p h e", h=HPC), Act.Copy)

            # per-chunk attention state carried across the pipeline.
            # PSUM quadrant rule (hardware): matmuls whose operands sit in PE
            # quadrant 0 (partitions 0:64) and quadrant 1 (64:128) must not
            # share a PSUM bank. Heads 0,2 read partitions 0:64 of their pair
            # tile, heads 1,3 read 64:128 — so each chunk uses TWO parity
            # tiles, each packing A (2x128) + out (2x65) + sinc (65) = 451 f32.
            # col offsets: A (stage1), out, sinc. The sinc region OVERLAPS A:
            # by the time stage2(c) writes sinc, the mask in stage1(c) has
            # fully consumed A (one pair earlier), so the bank fits in 386 f32.
            OA, OO, OSI = 0, 256, 0
            S_hist = {}    # c -> fp16 [128, 2, 65]: [:, par, :]=heads par,par+2
            pend = {}      # c -> (t_ev, t_od, a_sb)

            def head_slices(par, tpar):
                # heads with h%2 == par, their g=h//2 index
                return [(2 * g + par, g) for g in range(2)]

            def attn_stage2(c):
                """Chunk c's output block: S increment + AV + q@S_prev + norm.
                Emitted one pair behind stage1(c) so every cross-engine hop
                (tp -> phi_ks copy -> sinc, A -> mask -> AV) has a full pair
                of slack."""
                t_par, a_sb = pend.pop(c)
                last = c == NCHUNK - 1
                if not last:
                    # state increment S_c (not needed for the final chunk);
                    # head h=2g+par lands at partitions [64*par, 64*par+64),
                    # free slice g — so S rows match the qS stationary quadrant
                    for par in range(2):
                        hb = 64 * par
                        for h, g in head_slices(par, None):
                            nc.tensor.matmul(
                                t_par[par][hb:hb + 64,
                                           OSI + g * 65:OSI + (g + 1) * 65],
                                phi_ks[:, c, h * 64:(h + 1) * 64],
                                v_aug[:, c, h, :],
                                start=True, stop=True,
                            )
                    # running state in fp16 (one DVE op per parity tile)
                    sn = attn.tile([128, 2, 65], f16, tag="S", name=f"S_{c}",
                                   bufs=6)
                    for par in range(2):
                        hb = 64 * par
                        inc = (t_par[par][hb:hb + 64, OSI:OSI + 130]
                               .rearrange("p (g e) -> p g e", g=2))
                        if c == 0:
                            nc.vector.tensor_copy(sn[hb:hb + 64, :, :], inc)
                        else:
                            nc.vector.tensor_tensor(
                                sn[hb:hb + 64, :, :],
                                S_hist[c - 1][hb:hb + 64, :, :],
                                inc, op=Alu.add)
                    S_hist[c] = sn

                # AV + q@S_prev per head, immediately closing each PSUM
                # accumulation group (only one pending group per bank allowed)
                for par in range(2):
                    for h, g in head_slices(par, None):
                        o_sl = t_par[par][:, OO + g * 65:OO + (g + 1) * 65]
                        nc.tensor.matmul(
                            o_sl, a_sb[:, par, g, :], v_aug[:, c, h, :],
                            start=True, stop=(c == 0),
                        )
                        if c > 0:
                            hp, hb = h // 2, 64 * (h % 2)
                            nc.tensor.matmul(
                                o_sl,
                                phi_qT[hp][hb:hb + 64, c * 128:(c + 1) * 128],
                                S_hist[c - 1][hb:hb + 64, g, :],
                                start=False, stop=True,
                            )

                # normalize: strided reciprocal per parity + 4 scaled copies
                rcp = attn.tile([128, 2, 2], f32, tag="rcp", name=f"rcp_{c}",
                                bufs=6)
                o16 = attn.tile([128, HDC], f16, tag="o16", name=f"o16_{c}",
                                bufs=7)
                for par in range(2):
                    nc.vector.reciprocal(
                        rcp[:, par, :],
                        t_par[par][:, OO:OO + 130]
                        .rearrange("p (g e) -> p g e", g=2)[:, :, 64])
                # normalize: heads 0,1 on Act (Copy with per-partition
                # reciprocal scale), heads 2,3 on DVE except in the tail where
                # Act has run out of projection work (gpsimd can't read PSUM)
                for h in range(HPC):
                    par, g = h % 2, h // 2
                    src = t_par[par][:, OO + g * 65:OO + g * 65 + 64]
                    if h < 2 or (12 <= c < 15):
                        nc.scalar.activation(
                            o16[:, h * 64:(h + 1) * 64], src,
                            Act.Copy, scale=rcp[:, par, g:g + 1])
                    else:
                        nc.vector.tensor_scalar(
                            o16[:, h * 64:(h + 1) * 64], src,
                            rcp[:, par, g:g + 1], None, op0=Alu.mult)
                nc.sync.dma_start(out[c * 128:(c + 1) * 128, :], o16[:])

            def attn_stage1(c):
                # phi_k seq-major: transpose = matmul with identity as the
                # moving operand (fp32 PSUM out)
                tp = psum.tile([128, 2, 128], f32, tag="tp", name=f"tp_{c}",
                               bufs=1)
                for hp in range(2):
                    nc.tensor.matmul(
                        tp[:, hp, :], phi_kT[hp][:, c * 128:(c + 1) * 128],
                        ident[:], start=True, stop=True,
                    )
                nc.scalar.activation(phi_ks[:, c, :], tp[:], Act.Copy)

                t_par = [
                    psum.tile([128, 386], f32, tag=f"par{par}",
                              name=f"t{par}_{c}")
                    for par in range(2)
                ]
                for par in range(2):
                    for h, g in head_slices(par, None):
                        hp, hb = h // 2, 64 * (h % 2)
                        nc.tensor.matmul(
                            t_par[par][:, OA + g * 128:OA + (g + 1) * 128],
                            phi_kT[hp][hb:hb + 64, c * 128:(c + 1) * 128],
                            phi_qT[hp][hb:hb + 64, c * 128:(c + 1) * 128],
                            start=True, stop=True,
                        )
                # fused causal mask per parity (DVE — gpsimd can't read PSUM)
                a_sb = attn.tile([128, 2, 2, 128], f16, tag="Asb",
                                 name=f"a_sb_{c}", bufs=5)
                for par in range(2):
                    nc.vector.tensor_tensor(
                        a_sb[:, par, :, :],
                        t_par[par][:, 0:256].rearrange("p (g e) -> p g e", g=2),
                        maskT4[:, 0:2, :], op=Alu.mult)
                pend[c] = (t_par, a_sb)

            # software pipeline: attention stage units (stage1(c) = transposes
            # + A + mask, stage2(c) = sinc + AV + qS + norm) are interleaved
            # one-per-slot into the projection stream, each a few emission
            # blocks behind its phi / v_aug producers so no engine ever waits
            # on a cross-engine chain. The projection groups are uneven —
            # 4,4,4,3,1 chunks — so the final phi block is tiny and the
            # post-projection tail is short.
            s1, s2 = attn_stage1, attn_stage2
            V = emit_proj_v_chunk

            emit_proj("q", 0, 4)
            emit_proj("k", 0, 4)
            V(0); V(1); V(2); s1(0); V(3); s2(0)
            for c0 in (4, 8):
                b = c0 - 3  # first not-yet-emitted stage1 chunk
                emit_proj("q", c0, 4)
                s1(b); s2(b)
                emit_proj("k", c0, 4)
                s1(b + 1); s2(b + 1)
                V(c0); s1(b + 2)
                V(c0 + 1); s2(b + 2)
                V(c0 + 2); s1(b + 3)
                V(c0 + 3); s2(b + 3)
            emit_proj("q", 12, 3)
            s1(9); s2(9)
            emit_proj("k", 12, 3)
            s1(10); s2(10)
            V(12); s1(11)
            V(13); s2(11)
            V(14); s1(12)
            emit_proj("q", 15, 1)
            s2(12)
            emit_proj("k", 15, 1)
            s1(13)
            V(15)
            s2(13); s1(14); s2(14); s1(15); s2(15)

    nc.compile()
    return nc


def _get_nc():
    with _lock:
        if "nc" not in _cache:
            _cache["nc"] = _build_nc()
        return _cache["nc"]


def kernel(query, key, value, query_kernel, key_kernel, value_kernel):
    from concourse.bass_utils import run_bass_kernel_spmd

    nc = _get_nc()

    xT = {}
    for b in range(B):
        xT[("q", b)] = np.ascontiguousarray(query[b].T).astype(np.float16)
        xT[("k", b)] = np.ascontiguousarray(key[b].T).astype(np.float16)
        xT[("v", b)] = np.ascontiguousarray(value[b].T).astype(np.float16)

    in_maps = []
    for c in range(N_CORES):
        b, h0 = c // 4, 4 * (c % 4)
        in_maps.append({
            "xqT": xT[("q", b)],
            "xkT": xT[("k", b)],
            "xvT": xT[("v", b)],
            "wq": np.ascontiguousarray(
                query_kernel[:, h0:h0 + HPC, :].reshape(D, HDC)).astype(np.float16),
            "wk": np.ascontiguousarray(
                key_kernel[:, h0:h0 + HPC, :].reshape(D, HDC)).astype(np.float16),
            "wv": np.ascontiguousarray(
                value_kernel[:, h0:h0 + HPC, :].reshape(D, HDC)).astype(np.float16),
        })

    results = run_bass_kernel_spmd(nc, in_maps, core_ids=list(range(N_CORES)))

    # The reference ends with a FLAT reshape of [B*H, S, HD] -> (B, S, H*HD):
    # output rows [128h:128h+128] of batch b are head h's [S, HD] attention
    # output flat-reshaped to [128, H*HD].
    full = np.empty((B, S, H * HD), dtype=np.float32)
    for c in range(N_CORES):
        b, h0 = c // 4, 4 * (c % 4)
        av = results.results[c]["out"].astype(np.float32).reshape(S, HPC, HD)
        for hl in range(HPC):
            full[b, (h0 + hl) * 128:(h0 + hl + 1) * 128, :] = (
                av[:, hl, :].reshape(128, H * HD))
    return full


# revision 90
# speedup vs baseline: 1.0108x; 1.0108x over previous
"""Causal linear attention (elu+1 feature map) for Trainium2, 8 NeuronCores.

Problem: B=2, S=2048, D=1024, H=16, HD=64.
  q/k/v projections [S,D]@[D,H*HD], phi = elu+1, causal linear attention
  out[t] = (sum_{i<=t} (phi_q[t].phi_k[i]) v[i]) / (phi_q[t].sum_{i<=t} phi_k[i] + eps)

Sharding: core c -> (batch b=c//4, heads h0=4*(c%4) .. h0+3). No cross-core comm.
Host feeds x^T [D,S] per core in fp16 so the contraction dim d sits on SBUF
partitions with no on-chip transposes; all matmul operands are fp16 (fp32 PSUM
accumulate) which runs the PE at 1 cycle/row and halves HBM traffic.

Device algorithm (per core, 4 heads):
  - proj q,k -> phi_qT/phi_kT [64,2048] per head (head-pairs packed on 128
    partitions); v projected seq-major with an appended ones column whose
    matmul image is the softmax-free normalizer qz
  - phi_k seq layout via 128x128 PE transposes (matmul with identity moving
    operand, so the PSUM result stays fp32)
  - chunked attention, L=128, with two HW constraints found empirically:
    gpsimd cannot touch PSUM, and matmuls whose operands sit in different PE
    quadrants (partitions 0:64 vs 64:128) must not share a PSUM bank. Each
    chunk therefore uses two parity PSUM tiles [128, A 2x128 | out 2x65 with
    the sinc region overlapping A]: heads 0,2 (quadrant 0) in one, heads 1,3
    (quadrant 1) in the other. Causal mask is a fused tensor_tensor per
    parity on DVE; normalize = strided reciprocal (DVE) + scaled copies
    split between Act (activation Copy with per-partition scale) and DVE.
  - emission is software-pipelined: attention stage units interleave into
    the projection stream one-per-slot a few blocks behind their phi/v
    producers; projection groups are uneven (4,4,4,3,1 chunks) so the final
    phi block is tiny and the post-projection tail is ~one chunk.
"""

import threading

import numpy as np

B, S, D, H, HD = 2, 2048, 1024, 16, 64
N_CORES = 8
HPC = 4            # heads per core
HDC = HPC * HD     # 256 projected cols per core
NCHUNK = S // 128  # 16

_lock = threading.Lock()
_cache = {}


def _build_nc():
    import concourse.bass as bass
    import concourse.tile as tile
    from concourse import bacc, mybir

    f32 = mybir.dt.float32
    f16 = mybir.dt.float16
    Alu = mybir.AluOpType
    Act = mybir.ActivationFunctionType

    nc = bacc.Bacc("TRN2", target_bir_lowering=False, debug=False)

    xqT = nc.dram_tensor("xqT", [D, S], f16, kind="ExternalInput").ap()
    xkT = nc.dram_tensor("xkT", [D, S], f16, kind="ExternalInput").ap()
    xvT = nc.dram_tensor("xvT", [D, S], f16, kind="ExternalInput").ap()
    wq = nc.dram_tensor("wq", [D, HDC], f16, kind="ExternalInput").ap()
    wk = nc.dram_tensor("wk", [D, HDC], f16, kind="ExternalInput").ap()
    wv = nc.dram_tensor("wv", [D, HDC], f16, kind="ExternalInput").ap()
    out = nc.dram_tensor("out", [S, HDC], f16, kind="ExternalOutput").ap()

    DC = D // 128  # 8 contraction chunks

    with tile.TileContext(nc) as tc:
        with (
            tc.tile_pool(name="consts", bufs=1) as consts,
            tc.tile_pool(name="weights", bufs=1) as wpool,
            tc.tile_pool(name="resident", bufs=1) as res,
            tc.tile_pool(name="xin", bufs=1) as xin,
            tc.tile_pool(name="work", bufs=5) as work,
            tc.tile_pool(name="attn", bufs=3) as attn,
            tc.tile_pool(name="psum", bufs=2, space="PSUM") as psum,
        ):
            # ---- constants ----
            ones = consts.tile([128, 128], f16)
            nc.vector.memset(ones[:], 1.0)
            ident = consts.tile([128, 128], f16)
            nc.gpsimd.affine_select(
                ident[:], ones[:], pattern=[[-1, 128]], base=0,
                channel_multiplier=1, compare_op=Alu.is_equal, fill=0.0,
            )
            # causal mask in [j (part), t (free)] layout, 4 copies side by side
            maskT4 = consts.tile([128, 4, 128], f16)
            for i in range(4):
                nc.gpsimd.affine_select(
                    maskT4[:, i, :], ones[:], pattern=[[1, 128]], base=0,
                    channel_multiplier=-1, compare_op=Alu.is_ge, fill=0.0,
                )

            # ---- weights: [D, HDC] -> [128, DC, HDC] (partition = d % 128) ----
            # (DMA issued below, interleaved with the quarter-0 x loads)
            w_sb = {}
            w_dram = {"q": wq, "k": wk, "v": wv}
            for name in ("q", "k", "v"):
                w_sb[name] = wpool.tile([128, DC, HDC], f16, name=f"w{name}_sb")

            # ---- resident activations ----
            # head pairs hp=0 (heads 0,1) / hp=1 (heads 2,3), head at partition 64*(h%2)
            phi_qT = [res.tile([128, S], f16, name=f"phi_qT{i}") for i in range(2)]
            phi_kT = [res.tile([128, S], f16, name=f"phi_kT{i}") for i in range(2)]
            # seq-major: [s-in-chunk, (chunk, head, :)]
            phi_ks = res.tile([128, NCHUNK, HDC], f16, name="phi_ks")
            v_aug = res.tile([128, NCHUNK, HPC, 65], f16, name="v_aug")
            nc.vector.memset(v_aug[:, :, :, 64:65], 1.0)

            # ---- all input DMAs issued up front (SP queue), fp16 ----
            # x^T tiles: [128, DC, 512] per quarter, split in two for earlier
            # start; quarter 0 interleaved with the weight loads above is
            # handled by emission order (wq/xq0 first).
            xt = {}
            for qt in range(4):
                for tname, xdram in (("q", xqT), ("k", xkT), ("v", xvT)):
                    t = xin.tile([128, DC, 512], f16, name=f"x_{tname}_{qt}")
                    xt[(tname, qt)] = t

            def load_x(tname, qt, nsplit):
                t = xt[(tname, qt)]
                xdram = {"q": xqT, "k": xkT, "v": xvT}[tname]
                src = xdram.rearrange("(dc p) s -> p dc s", p=128)
                step = DC // nsplit
                for i in range(nsplit):
                    nc.sync.dma_start(
                        t[:, i * step:(i + 1) * step, :],
                        src[:, i * step:(i + 1) * step,
                            qt * 512:(qt + 1) * 512])

            # weight loads interleaved with quarter-0 x loads, dc-chunk by
            # dc-chunk for tensor "q" so the first projection matmul can start
            # as early as possible
            wsrc = {t: w_dram[t].rearrange("(dc p) m -> p dc m", p=128)
                    for t in ("q", "k", "v")}
            xsrc = {t: d.rearrange("(dc p) s -> p dc s", p=128)
                    for t, d in (("q", xqT), ("k", xkT), ("v", xvT))}
            for i in range(4):
                sl = slice(2 * i, 2 * i + 2)
                nc.sync.dma_start(w_sb["q"][:, sl, :], wsrc["q"][:, sl, :])
                nc.sync.dma_start(xt[("q", 0)][:, sl, :],
                                  xsrc["q"][:, sl, 0:512])
            for tname in ("k", "v"):
                nc.sync.dma_start(w_sb[tname][:, 0:DC // 2, :],
                                  wsrc[tname][:, 0:DC // 2, :])
                nc.sync.dma_start(w_sb[tname][:, DC // 2:DC, :],
                                  wsrc[tname][:, DC // 2:DC, :])
                load_x(tname, 0, 4)
            for qt in range(1, 4):
                for tname in ("q", "k", "v"):
                    load_x(tname, qt, 2)

            def phi_from_psum(ps, dst, w):
                # phi(x) = exp(min(x,0)) + max(x,0); m' = relu(-x); e = exp(-m')
                t1 = work.tile([128, 512], f32, tag="phi1")
                t2 = work.tile([128, 512], f16, tag="phi2")
                nc.scalar.activation(t1[:, 0:w], ps[:, 0:w],
                                     Act.Relu, scale=-1.0)
                nc.scalar.activation(t2[:, 0:w], t1[:, 0:w],
                                     Act.Exp, scale=-1.0)
                nc.vector.scalar_tensor_tensor(
                    dst, ps[:, 0:w], 0.0, t2[:, 0:w],
                    op0=Alu.max, op1=Alu.add)

            def emit_proj(tname, c0, n):
                # project chunks [c0, c0+n) of tensor q/k (all within one
                # quarter's resident x tile)
                qt, cc0 = c0 // 4, (c0 % 4) * 128
                w = n * 128
                dst = {"q": phi_qT, "k": phi_kT}[tname]
                x = xt[(tname, qt)]
                for hp in range(2):
                    ps = psum.tile([128, 512], f32, tag="proj",
                                   name=f"ps_{tname}_{c0}_{hp}", bufs=3)
                    for dc in range(DC):
                        nc.tensor.matmul(
                            ps[:, 0:w],
                            w_sb[tname][:, dc, hp * 128:(hp + 1) * 128],
                            x[:, dc, cc0:cc0 + w],
                            start=(dc == 0), stop=(dc == DC - 1),
                        )
                    phi_from_psum(ps, dst[hp][:, c0 * 128:c0 * 128 + w], w)

            def emit_proj_v_chunk(c):
                qt, cc = c // 4, c % 4
                xv = xt[("v", qt)]
                ps = psum.tile([128, 512], f32, tag="proj", name=f"ps_v_{c}",
                               bufs=3)
                for dc in range(DC):
                    nc.tensor.matmul(
                        ps[:, 0:HDC], xv[:, dc, cc * 128:(cc + 1) * 128],
                        w_sb["v"][:, dc, :],
                        start=(dc == 0), stop=(dc == DC - 1),
                    )
                nc.scalar.activation(
                    v_aug[:, c, :, 0:64],
                    ps[:, 0:HDC].rearrange("p (h e) -> p h e", h=HPC), Act.Copy)

            # per-chunk attention state carried across the pipeline.
            # PSUM quadrant rule (hardware): matmuls whose operands sit in PE
            # quadrant 0 (partitions 0:64) and quadrant 1 (64:128) must not
            # share a PSUM bank. Heads 0,2 read partitions 0:64 of their pair
            # tile, heads 1,3 read 64:128 — so each chunk uses TWO parity
            # tiles, each packing A (2x128) + out (2x65) + sinc (65) = 451 f32.
            # col offsets: A (stage1), out, sinc. The sinc region OVERLAPS A:
            # by the time stage2(c) writes sinc, the mask in stage1(c) has
            # fully consumed A (one pair earlier), so the bank fits in 386 f32.
            OA, OO, OSI = 0, 256, 0
            S_hist = {}    # c -> fp16 [128, 2, 65]: [:, par, :]=heads par,par+2
            pend = {}      # c -> (t_ev, t_od, a_sb)

            def head_slices(par, tpar):
                # heads with h%2 == par, their g=h//2 index
                return [(2 * g + par, g) for g in range(2)]

            def attn_stage2(c):
                """Chunk c's output block: S increment + AV + q@S_prev + norm.
                Emitted one pair behind stage1(c) so every cross-engine hop
                (tp -> phi_ks copy -> sinc, A -> mask -> AV) has a full pair
                of slack."""
                t_par, a_sb = pend.pop(c)
                last = c == NCHUNK - 1
                if not last:
                    # state increment S_c (not needed for the final chunk);
                    # head h=2g+par lands at partitions [64*par, 64*par+64),
                    # free slice g — so S rows match the qS stationary quadrant
                    for par in range(2):
                        hb = 64 * par
                        for h, g in head_slices(par, None):
                            nc.tensor.matmul(
                                t_par[par][hb:hb + 64,
                                           OSI + g * 65:OSI + (g + 1) * 65],
                                phi_ks[:, c, h * 64:(h + 1) * 64],
                                v_aug[:, c, h, :],
                                start=True, stop=True,
                            )
                    # running state in fp16 (one DVE op per parity tile)
                    sn = attn.tile([128, 2, 65], f16, tag="S", name=f"S_{c}",
                                   bufs=6)
                    for par in range(2):
                        hb = 64 * par
                        inc = (t_par[par][hb:hb + 64, OSI:OSI + 130]
                               .rearrange("p (g e) -> p g e", g=2))
                        if c == 0:
                            nc.vector.tensor_copy(sn[hb:hb + 64, :, :], inc)
                        else:
                            nc.vector.tensor_tensor(
                                sn[hb:hb + 64, :, :],
                                S_hist[c - 1][hb:hb + 64, :, :],
                                inc, op=Alu.add)
                    S_hist[c] = sn

                # AV + q@S_prev per head, immediately closing each PSUM
                # accumulation group (only one pending group per bank allowed)
                for par in range(2):
                    for h, g in head_slices(par, None):
                        o_sl = t_par[par][:, OO + g * 65:OO + (g + 1) * 65]
                        nc.tensor.matmul(
                            o_sl, a_sb[:, par, g, :], v_aug[:, c, h, :],
                            start=True, stop=(c == 0),
                        )
                        if c > 0:
                            hp, hb = h // 2, 64 * (h % 2)
                            nc.tensor.matmul(
                                o_sl,
                                phi_qT[hp][hb:hb + 64, c * 128:(c + 1) * 128],
                                S_hist[c - 1][hb:hb + 64, g, :],
                                start=False, stop=True,
                            )

                # normalize: strided reciprocal per parity + 4 scaled copies
                rcp = attn.tile([128, 2, 2], f32, tag="rcp", name=f"rcp_{c}",
                                bufs=6)
                o16 = attn.tile([128, HDC], f16, tag="o16", name=f"o16_{c}",
                                bufs=7)
                for par in range(2):
                    nc.vector.reciprocal(
                        rcp[:, par, :],
                        t_par[par][:, OO:OO + 130]
                        .rearrange("p (g e) -> p g e", g=2)[:, :, 64])
                # normalize: heads 0,1 on Act (Copy with per-partition
                # reciprocal scale), heads 2,3 on DVE except in the tail where
                # Act has run out of projection work (gpsimd can't read PSUM)
                for h in range(HPC):
                    par, g = h % 2, h // 2
                    src = t_par[par][:, OO + g * 65:OO + g * 65 + 64]
                    if h < 2 or (12 <= c < 15):
                        nc.scalar.activation(
                            o16[:, h * 64:(h + 1) * 64], src,
                            Act.Copy, scale=rcp[:, par, g:g + 1])
                    else:
                        nc.vector.tensor_scalar(
                            o16[:, h * 64:(h + 1) * 64], src,
                            rcp[:, par, g:g + 1], None, op0=Alu.mult)
                nc.sync.dma_start(out[c * 128:(c + 1) * 128, :], o16[:])

            def attn_stage1(c):
                # phi_k seq-major: transpose = matmul with identity as the
                # moving operand (fp32 PSUM out)
                tp = psum.tile([128, 2, 128], f32, tag="tp", name=f"tp_{c}",
                               bufs=1)
                for hp in range(2):
                    nc.tensor.matmul(
                        tp[:, hp, :], phi_kT[hp][:, c * 128:(c + 1) * 128],
                        ident[:], start=True, stop=True,
                    )
                nc.scalar.activation(phi_ks[:, c, :], tp[:], Act.Copy)

                t_par = [
                    psum.tile([128, 386], f32, tag=f"par{par}",
                              name=f"t{par}_{c}")
                    for par in range(2)
                ]
                for par in range(2):
                    for h, g in head_slices(par, None):
                        hp, hb = h // 2, 64 * (h % 2)
                        nc.tensor.matmul(
                            t_par[par][:, OA + g * 128:OA + (g + 1) * 128],
                            phi_kT[hp][hb:hb + 64, c * 128:(c + 1) * 128],
                            phi_qT[hp][hb:hb + 64, c * 128:(c + 1) * 128],
                            start=True, stop=True,
                        )
                # fused causal mask per parity (DVE — gpsimd can't read PSUM)
                a_sb = attn.tile([128, 2, 2, 128], f16, tag="Asb",
                                 name=f"a_sb_{c}", bufs=5)
                for par in range(2):
                    nc.vector.tensor_tensor(
                        a_sb[:, par, :, :],
                        t_par[par][:, 0:256].rearrange("p (g e) -> p g e", g=2),
                        maskT4[:, 0:2, :], op=Alu.mult)
                pend[c] = (t_par, a_sb)

            # software pipeline: attention stage units (stage1(c) = transposes
            # + A + mask, stage2(c) = sinc + AV + qS + norm) are interleaved
            # one-per-slot into the projection stream, each a few emission
            # blocks behind its phi / v_aug producers so no engine ever waits
            # on a cross-engine chain. The projection groups are uneven —
            # 4,4,4,3,1 chunks — so the final phi block is tiny and the
            # post-projection tail is short.
            s1, s2 = attn_stage1, attn_stage2
            V = emit_proj_v_chunk

            emit_proj("q", 0, 4)
            emit_proj("k", 0, 4)
            V(0); V(1); V(2); s1(0); V(3); s2(0)
            for c0 in (4, 8):
                b = c0 - 3  # first not-yet-emitted stage1 chunk
                emit_proj("q", c0, 4)
                s1(b); s2(b)
                emit_proj("k", c0, 4)
                s1(b + 1); s2(b + 1)
                V(c0); s1(b + 2)
                V(c0 + 1); s2(b + 2)
                V(c0 + 2); s1(b + 3)
                V(c0 + 3); s2(b + 3)
            emit_proj("q", 12, 2)
            s1(9); s2(9)
            emit_proj("k", 12, 2)
            s1(10); s2(10)
            V(12); s1(11)
            V(13); s2(11)
            emit_proj("q", 14, 2)
            s1(12); s2(12)
            emit_proj("k", 14, 2)
            s1(13)
            V(14); s2(13)
            V(15)
            s1(14); s2(14); s1(15); s2(15)

    nc.compile()
    return nc


def _get_nc():
    with _lock:
        if "nc" not in _cache:
            _cache["nc"] = _build_nc()
        return _cache["nc"]


def kernel(query, key, value, query_kernel, key_kernel, value_kernel):
    from concourse.bass_utils import run_bass_kernel_spmd

    nc = _get_nc()

    xT = {}
    for b in range(B):
        xT[("q", b)] = np.ascontiguousarray(query[b].T).astype(np.float16)
        xT[("k", b)] = np.ascontiguousarray(key[b].T).astype(np.float16)
        xT[("v", b)] = np.ascontiguousarray(value[b].T).astype(np.float16)

    in_maps = []
    for c in range(N_CORES):
        b, h0 = c // 4, 4 * (c % 4)
        in_maps.append({
            "xqT": xT[("q", b)],
            "xkT": xT[("k", b)],
            "xvT": xT[("v", b)],
            "wq": np.ascontiguousarray(
                query_kernel[:, h0:h0 + HPC, :].reshape(D, HDC)).astype(np.float16),
            "wk": np.ascontiguousarray(
                key_kernel[:, h0:h0 + HPC, :].reshape(D, HDC)).astype(np.float16),
            "wv": np.ascontiguousarray(
                value_kernel[:, h0:h0 + HPC, :].reshape(D, HDC)).astype(np.float16),
        })

    results = run_bass_kernel_spmd(nc, in_maps, core_ids=list(range(N_CORES)))

    # The reference ends with a FLAT reshape of [B*H, S, HD] -> (B, S, H*HD):
    # output rows [128h:128h+128] of batch b are head h's [S, HD] attention
    # output flat-reshaped to [128, H*HD].
    full = np.empty((B, S, H * HD), dtype=np.float32)
    for c in range(N_CORES):
        b, h0 = c // 4, 4 * (c % 4)
        av = results.results[c]["out"].astype(np.float32).reshape(S, HPC, HD)
        for hl in range(HPC):
            full[b, (h0 + hl) * 128:(h0 + hl + 1) * 128, :] = (
                av[:, hl, :].reshape(128, H * HD))
    return full
